# revision 16
# baseline (speedup 1.0000x reference)
"""Multi-head attention Trainium2 Bass kernel (nn_MultiHeadAttention_69655779607087).

Problem (hardcoded): B=4, L=2048, D_MODEL=1024, H=16, D_QK=D_V=64, fp32.
    q = einsum('bld,hdk->bhlk', x_query, Wq); k,v likewise
    scores = q @ k^T / 8 ; attn = softmax(scores); heads = attn @ v
    out = concat_heads(heads) @ Wout          -> [B, L, D_MODEL]

Sharding (8 cores, no collectives): core c handles batch b=c//2 and query
half h=c%2 (1024 query tokens). K/V projections for batch b are computed
redundantly by the 2 cores sharing the batch; everything else is perfectly
sharded. Host slices/transposes/casts inputs per core and concatenates the
8 [1024, 1024] fp32 output shards.

v2 design (matmul operands bf16, PSUM accumulation fp32):
  - Projections phased per head-half (pairs 0-3, then 4-7), with chunk-
    resident x tiles, so attention of half 0 overlaps projections of half 1.
  - Scores per head-pair are two concurrent row-tiled K=64 matmuls (heads of
    a pair live on partition halves of the same KT/QT 128-block), writing
    separate PSUM banks. No zero-padding waste.
  - exp is the only ScalarE work (it is the critical engine: 33.5M
    activations/core). ACTIVATE granularity [128, 2, 512] (one (pair, s,
    qh) group).
  - attn@V accumulates per head with the ones-augmented V (extra output row
    = softmax denominator), 16 s-blocks per (pair, qh) pass.
  - Softmax epilogue per (pair, qh): DVE extracts the two denominator rows,
    a SBUF->SBUF DMA spreads them over 16 partitions, one parallel DVE
    reciprocal, DMA collapse + partition-broadcast, DVE mul -> normalized
    heads^T in SBUF.
  - PSUM budget: sp 2 tiles x 2 banks + op 2 banks + proj 2 x 1 bank = 8.
"""

import os
import sys

for _p in ("/opt/trn_rl_repo", "/opt/pypackages"):
    if _p not in sys.path:
        sys.path.append(_p)

import numpy as np

H, D, DK, DV = 16, 1024, 64, 64
B, L = 4, 2048
LQ = 1024  # query tokens per core
P = 128
NKB = D // P  # 8 contraction blocks over d_model
NHB = (H * DK) // P  # 8 head-pair blocks
NSB = L // P  # 16 key-token blocks
NMQ = LQ // P  # 8 query-token blocks

_CACHE = {}


def _build_bass():
    import concourse.bass as bass
    import concourse.tile as tile
    from concourse import mybir
    from concourse.bass import ts

    f32 = mybir.dt.float32
    bf16 = mybir.dt.bfloat16
    EXP = mybir.ActivationFunctionType.Exp

    nc = bass.Bass()
    # host-prepped, bf16:
    xqT = nc.dram_tensor("xqt", [D, LQ], bf16, kind="ExternalInput")
    xkT = nc.dram_tensor("xkt", [D, L], bf16, kind="ExternalInput")
    # xvT tiled [k, mg, 128, m8, 128] : per (k, mg) one [128, 8, 128] row tile
    xvT = nc.dram_tensor("xvt", [NKB, 2, P, 8, P], bf16, kind="ExternalInput")
    # wq/wk tiled [k, 128, m, 128] : per k one [128, 8, 128] row tile
    wq = nc.dram_tensor("wq", [NKB, P, NHB, P], bf16, kind="ExternalInput")
    wk = nc.dram_tensor("wk", [NKB, P, NHB, P], bf16, kind="ExternalInput")
    wv = nc.dram_tensor("wv", [D, H * DV], bf16, kind="ExternalInput")
    wout = nc.dram_tensor("wout", [H * DV, D], bf16, kind="ExternalInput")
    out = nc.dram_tensor("out", [LQ, D], f32, kind="ExternalOutput")

    lp = nc.allow_low_precision(
        reason="bf16 matmul operands; accumulation stays fp32 in PSUM"
    )
    lp.__enter__()
    with tile.TileContext(nc) as tc:
        with (
            tc.tile_pool(name="persist", bufs=1) as persist,
            tc.tile_pool(name="xin", bufs=2) as xin,
            tc.tile_pool(name="attn", bufs=6) as attn_pool,
            tc.tile_pool(name="small", bufs=1) as small,
            tc.tile_pool(name="outp", bufs=3) as outp,
            tc.tile_pool(name="psum", bufs=1, space="PSUM") as psum,
            tc.tile_pool(name="dramp", bufs=2, space="DRAM") as dramp,
        ):
            # ---- persistent SBUF tensors (bf16) ----
            QT = persist.tile([P, NHB, LQ], bf16)  # 16 KB/part
            KT = persist.tile([P, NHB, L], bf16)  # 32 KB/part
            VA = persist.tile([P, NSB, H, DV + 1], bf16)  # V_aug, 32.5 KB/part
            HT = persist.tile([P, NHB, LQ], bf16)  # heads^T, 16 KB/part
            WQ = persist.tile([P, NKB, NHB, P], bf16)  # 16 KB/part
            WK = persist.tile([P, NKB, NHB, P], bf16)  # 16 KB/part
            WV = persist.tile([P, NKB, H * DV], bf16)  # 16 KB/part
            WO = persist.tile([P, NHB, D], bf16)  # 16 KB/part
            # Keep the sync queue free for the first xk chunk (K-proj is the
            # startup critical path): weights go on the scalar-engine HWDGE
            # queue (idle at startup) and the gpsimd SWDGE queue.
            for k in range(NKB):
                nc.gpsimd.dma_start(out=WV[:, k], in_=wv[ts(k, P), :])
            for k in range(NKB):
                nc.scalar.dma_start(out=WK[:, k], in_=wk[k])
            for k in range(NKB):
                nc.scalar.dma_start(out=WQ[:, k], in_=wq[k])
            for k in range(NKB):
                nc.gpsimd.dma_start(out=WO[:, k], in_=wout[ts(k, P), :])
            # ones column of V_aug: single strided memset
            nc.gpsimd.memset(VA[:, :, :, DV : DV + 1], 1.0)

            def k_proj(half):
                """KT[:, m, :] for pairs of `half` (m in 4*half..4*half+3)."""
                for nh in range(4):  # tok chunks of 512
                    xk = xin.tile([P, NKB, 512], bf16, tag="xk")
                    for k in range(NKB):
                        nc.sync.dma_start(
                            out=xk[:, k], in_=xkT[ts(k, P), ts(nh, 512)]
                        )
                    for m in range(4 * half, 4 * half + 4):
                        pt = psum.tile([P, 512], f32, tag="proj", bufs=2)
                        for k in range(NKB):
                            nc.tensor.matmul(
                                pt[:, :],
                                lhsT=WK[:, k, m, :],
                                rhs=xk[:, k, :],
                                start=(k == 0),
                                stop=(k == NKB - 1),
                            )
                        nc.vector.tensor_copy(KT[:, m, ts(nh, 512)], pt[:, :])

            def q_proj(half):
                for nh in range(2):  # tok chunks of 512
                    xq = xin.tile([P, NKB, 512], bf16, tag="xk")
                    for k in range(NKB):
                        nc.sync.dma_start(
                            out=xq[:, k], in_=xqT[ts(k, P), ts(nh, 512)]
                        )
                    for m in range(4 * half, 4 * half + 4):
                        pt = psum.tile([P, 512], f32, tag="proj", bufs=2)
                        for k in range(NKB):
                            nc.tensor.matmul(
                                pt[:, :],
                                lhsT=WQ[:, k, m, :],
                                rhs=xq[:, k, :],
                                start=(k == 0),
                                stop=(k == NKB - 1),
                            )
                        nc.vector.tensor_copy(QT[:, m, ts(nh, 512)], pt[:, :])

            def kq_proj_m(w_res, x_dram, dst, n_chunk, m):
                """Project one head-pair block m (x chunks re-streamed)."""
                for nh in range(n_chunk):
                    xt = xin.tile([P, NKB, 512], bf16, tag="xk")
                    for k in range(NKB):
                        nc.sync.dma_start(
                            out=xt[:, k], in_=x_dram[ts(k, P), ts(nh, 512)]
                        )
                    pt = psum.tile([P, 512], f32, tag="proj", bufs=2)
                    for k in range(NKB):
                        nc.tensor.matmul(
                            pt[:, :],
                            lhsT=w_res[:, k, m, :],
                            rhs=xt[:, k, :],
                            start=(k == 0),
                            stop=(k == NKB - 1),
                        )
                    nc.vector.tensor_copy(dst[:, m, ts(nh, 512)], pt[:, :])

            def v_proj(half):
                """VA[:, :, 8*half:8*half+8, 0:64] (8 heads of `half`)."""
                for mgq in range(4):  # tok-block groups of 4
                    mg, sub = divmod(mgq, 2)
                    xv = xin.tile([P, NKB, 4, P], bf16, tag="xk")
                    for k in range(NKB):
                        nc.sync.dma_start(
                            out=xv[:, k], in_=xvT[k, mg][:, sub * 4 : sub * 4 + 4, :]
                        )
                    for m4 in range(4):
                        pt = psum.tile([P, 512], f32, tag="proj", bufs=2)
                        for k in range(NKB):
                            nc.tensor.matmul(
                                pt[:, :],
                                lhsT=xv[:, k, m4, :],
                                rhs=WV[:, k, ts(half, 512)],
                                start=(k == 0),
                                stop=(k == NKB - 1),
                            )
                        nc.vector.tensor_copy(
                            VA[:, mgq * 4 + m4, half * 8 : half * 8 + 8, 0:DV],
                            pt.rearrange("p (h v) -> p h v", h=8),
                        )

            def attention(hb, qh):
                """One head pair: heads 2hb (partitions 0-63 of block hb) and
                2hb+1 (partitions 64-127), all 16 s-blocks, one q half."""
                if True:
                    opt = psum.tile([P, 2, 512], f32, tag="op", bufs=1)
                    for s in range(NSB):
                        sp = psum.tile([P, 2, 512], f32, tag="sp", bufs=2)
                        # two concurrent row-tiled K=64 matmuls (head pair)
                        nc.tensor.matmul(
                            sp[:, 0, :],
                            lhsT=KT[0:DK, hb, ts(s, P)],
                            rhs=QT[0:DK, hb, ts(qh, 512)],
                            start=True,
                            stop=True,
                        )
                        nc.tensor.matmul(
                            sp[:, 1, :],
                            lhsT=KT[DK:P, hb, ts(s, P)],
                            rhs=QT[DK:P, hb, ts(qh, 512)],
                            start=True,
                            stop=True,
                        )
                        ae = attn_pool.tile([P, 2, 512], bf16, tag="ae")
                        nc.scalar.activation(
                            out=ae[:, :, :], in_=sp[:, :, :], func=EXP, scale=0.125
                        )
                        for j in range(2):
                            nc.tensor.matmul(
                                opt[0 : DV + 1, j, :],
                                lhsT=VA[:, s, 2 * hb + j, :],
                                rhs=ae[:, j, :],
                                start=(s == 0),
                                stop=(s == NSB - 1),
                            )
                    # ---- softmax epilogue for this (pair, qh) ----
                    # Free `opt` fast (it is single-buffered): extract the
                    # denominator rows (fp32) and the unnormalized heads
                    # (bf16), then normalize lazily off the critical path.
                    den = small.tile([1, 2, 512], f32, tag="den")
                    nc.vector.tensor_copy(den[:, :, :], opt[DV : DV + 1, :, :])
                    htu = small.tile([DV, 2, 512], bf16, tag="htu", bufs=2)
                    nc.vector.tensor_copy(htu[:, :, :], opt[0:DV, :, :])
                    dden = dramp.tile(
                        [1, 2, 512], f32, tag="dden", name=f"dden_{hb}_{qh}"
                    )
                    nc.gpsimd.dma_start(out=dden[:, :, :], in_=den[:, :, :])
                    den16 = small.tile([16, DV], f32, tag="den16")
                    nc.gpsimd.dma_start(
                        out=den16[:, :],
                        in_=dden.rearrange("p h (j c) -> p (h j) c", j=8)[0],
                    )
                    r16 = small.tile([16, DV], f32, tag="r16")
                    nc.vector.reciprocal(r16[:, :], den16[:, :])
                    rb16 = small.tile([16, DV], bf16, tag="rb16")
                    nc.vector.tensor_copy(rb16[:, :], r16[:, :])
                    rden = dramp.tile(
                        [1, 2, 512], bf16, tag="rden", name=f"rden_{hb}_{qh}"
                    )
                    nc.gpsimd.dma_start(
                        out=rden.rearrange("p h (j c) -> p (h j) c", j=8)[0],
                        in_=rb16[:, :],
                    )
                    rcb = small.tile([DV, 2, 512], bf16, tag="rcb")
                    nc.gpsimd.dma_start(
                        out=rcb[:, :, :],
                        in_=rden[0:1, :, :].to_broadcast((DV, 2, 512)),
                    )
                    for j in range(2):
                        nc.vector.tensor_mul(
                            HT[j * DK : j * DK + DK, hb, ts(qh, 512)],
                            htu[:, j, :],
                            rcb[:, j, :],
                        )

            # ---- phased schedule ----
            # attention(0) is emitted before v_proj(0): its scores/exp only
            # need KT/QT, so exp starts ~45us earlier; its attn@V matmuls
            # block on VA and the scheduler fills with v_proj work.
            k_proj(0)
            q_proj(0)
            v_proj(0)
            attention(0, 0)
            # half-1 projections: emitted here (before the hb>=4 attention
            # passes -- the VA dependency tracker needs v_proj writes emitted
            # before the attn@V reads) but DEPRIORITIZED so the scheduler
            # prefers the exp critical chain and uses projections as filler.
            _save_pri = tc.cur_priority
            tc.cur_priority = 10_000_000
            k_proj(1)
            q_proj(1)
            v_proj(1)
            tc.cur_priority = _save_pri
            # all qh=0 passes first: the first half of the out-projection
            # becomes ready mid-kernel and fills PE spare during qh=1 passes.
            for hb in range(1, NHB):
                attention(hb, 0)
            for hb in range(NHB):
                attention(hb, 1)

            # ---- out projection ----
            for nh in range(2):  # dm halves
                for m in range(NMQ):
                    pt = psum.tile([P, 512], f32, tag="proj", bufs=2, name=f"po_{nh}_{m}")
                    for hb in range(NHB):
                        nc.tensor.matmul(
                            pt[:, :],
                            lhsT=HT[:, hb, ts(m, P)],
                            rhs=WO[:, hb, ts(nh, 512)],
                            start=(hb == 0),
                            stop=(hb == NHB - 1),
                        )
                    ot = outp.tile([P, 512], f32, tag="ot", name=f"ot_{nh}_{m}")
                    nc.vector.tensor_copy(ot, pt[:, :])
                    (nc.gpsimd if m % 2 == 0 else nc.sync).dma_start(
                        out=out[ts(m, P), ts(nh, 512)], in_=ot
                    )
    lp.__exit__(None, None, None)

    _split_multi_waits(nc)
    return nc


def _split_multi_waits(nc, max_waits: int = 1):
    """Walrus's setupSyncWait rejects instructions carrying more than a
    struct-specific number of sync waits (e.g. the Tile kernel-tail Drain
    gathers one wait per live semaphore). Hoist excess waits into prepended
    single-wait NoOps on the same engine."""
    from concourse import mybir

    for f in nc.m.functions:
        for blk in f.blocks:
            out = []
            for inst in blk.instructions:
                si = inst.sync_info
                waits = list(si.on_wait) if (si is not None and si.on_wait) else []
                if len(waits) > max_waits:
                    keep = waits[-max_waits:]
                    for w in waits[:-max_waits]:
                        nop = mybir.InstNoOp(
                            name=nc.get_next_instruction_name(),
                            ins=[],
                            outs=[],
                            sync_info=mybir.SyncInfo(on_wait=[w], on_update=[]),
                        )
                        nop.engine = inst.engine
                        try:
                            nop.bass_nofuse = True
                        except Exception:
                            pass
                        nc.register_instruction(nop)
                        out.append(nop)
                    si.on_wait = keep
                out.append(inst)
            blk.instructions = out


def _get_nc():
    if "nc" not in _CACHE:
        _CACHE["nc"] = _build_bass()
    return _CACHE["nc"]


def _prep_in_maps(x_query, x_key, x_value, Wq, Wk, Wv, Wout):
    import ml_dtypes

    bf = ml_dtypes.bfloat16
    x_query = np.asarray(x_query, dtype=np.float32)
    x_key = np.asarray(x_key, dtype=np.float32)
    x_value = np.asarray(x_value, dtype=np.float32)
    # [H, D, dk] -> [D, H*dk]
    wq_cat = np.asarray(Wq, np.float32).transpose(1, 0, 2).reshape(D, H * DK)
    wk_cat = np.asarray(Wk, np.float32).transpose(1, 0, 2).reshape(D, H * DK)
    wv_cat = np.ascontiguousarray(
        np.asarray(Wv, np.float32).transpose(1, 0, 2).reshape(D, H * DV)
    ).astype(bf)
    # wq/wk into [k, 128, m, 128] (contiguous [m,128] per (k,p) row)
    wq_t = np.ascontiguousarray(wq_cat.reshape(NKB, P, NHB, P)).astype(bf)
    wk_t = np.ascontiguousarray(wk_cat.reshape(NKB, P, NHB, P)).astype(bf)
    wout_c = np.ascontiguousarray(np.asarray(Wout, np.float32)).astype(bf)

    in_maps = []
    for c in range(8):
        b, half = divmod(c, 2)
        xq_sh = np.ascontiguousarray(
            x_query[b, half * LQ : (half + 1) * LQ, :].T
        ).astype(bf)  # [D, LQ]
        xk_sh = np.ascontiguousarray(x_key[b].T).astype(bf)  # [D, L]
        xvT_full = x_value[b].T  # [D, L]
        # [k, mg, 128, m8, 128]
        xv_t = np.ascontiguousarray(
            xvT_full.reshape(NKB, P, 2, 8, P).transpose(0, 2, 1, 3, 4)
        ).astype(bf)
        in_maps.append(
            {
                "xqt": xq_sh,
                "xkt": xk_sh,
                "xvt": xv_t,
                "wq": wq_t,
                "wk": wk_t,
                "wv": wv_cat,
                "wout": wout_c,
            }
        )
    return in_maps


def kernel(x_query, x_key, x_value, Wq, Wk, Wv, Wout):
    from concourse.bass_utils import run_bass_kernel_spmd

    nc = _get_nc()
    in_maps = _prep_in_maps(x_query, x_key, x_value, Wq, Wk, Wv, Wout)
    trace = bool(int(os.environ.get("MHA_TRACE", "0")))
    res = run_bass_kernel_spmd(nc, in_maps, list(range(8)), trace=trace)
    _CACHE["last_result"] = res
    out = np.empty((B, L, D), np.float32)
    for c in range(8):
        b, half = divmod(c, 2)
        out[b, half * LQ : (half + 1) * LQ, :] = res.results[c]["out"]
    return out


# revision 17
# speedup vs baseline: 1.0525x; 1.0525x over previous
"""Multi-head attention Trainium2 Bass kernel (nn_MultiHeadAttention_69655779607087).

Problem (hardcoded): B=4, L=2048, D_MODEL=1024, H=16, D_QK=D_V=64, fp32.
    q = einsum('bld,hdk->bhlk', x_query, Wq); k,v likewise
    scores = q @ k^T / 8 ; attn = softmax(scores); heads = attn @ v
    out = concat_heads(heads) @ Wout          -> [B, L, D_MODEL]

Sharding (8 cores, no collectives): core c handles batch b=c//2 and query
half h=c%2 (1024 query tokens). K/V projections for batch b are computed
redundantly by the 2 cores sharing the batch; everything else is perfectly
sharded. Host slices/transposes/casts inputs per core and concatenates the
8 [1024, 1024] fp32 output shards.

v2 design (matmul operands bf16, PSUM accumulation fp32):
  - Projections phased per head-half (pairs 0-3, then 4-7), with chunk-
    resident x tiles, so attention of half 0 overlaps projections of half 1.
  - Scores per head-pair are two concurrent row-tiled K=64 matmuls (heads of
    a pair live on partition halves of the same KT/QT 128-block), writing
    separate PSUM banks. No zero-padding waste.
  - exp is the only ScalarE work (it is the critical engine: 33.5M
    activations/core). ACTIVATE granularity [128, 2, 512] (one (pair, s,
    qh) group).
  - attn@V accumulates per head with the ones-augmented V (extra output row
    = softmax denominator), 16 s-blocks per (pair, qh) pass.
  - Softmax epilogue per (pair, qh): DVE extracts the two denominator rows,
    a SBUF->SBUF DMA spreads them over 16 partitions, one parallel DVE
    reciprocal, DMA collapse + partition-broadcast, DVE mul -> normalized
    heads^T in SBUF.
  - PSUM budget: sp 2 tiles x 2 banks + op 2 banks + proj 2 x 1 bank = 8.
"""

import os
import sys

for _p in ("/opt/trn_rl_repo", "/opt/pypackages"):
    if _p not in sys.path:
        sys.path.append(_p)

import numpy as np

H, D, DK, DV = 16, 1024, 64, 64
B, L = 4, 2048
LQ = 1024  # query tokens per core
P = 128
NKB = D // P  # 8 contraction blocks over d_model
NHB = (H * DK) // P  # 8 head-pair blocks
NSB = L // P  # 16 key-token blocks
NMQ = LQ // P  # 8 query-token blocks

_CACHE = {}


def _build_bass():
    import concourse.bass as bass
    import concourse.tile as tile
    from concourse import mybir
    from concourse.bass import ts

    f32 = mybir.dt.float32
    bf16 = mybir.dt.bfloat16
    EXP = mybir.ActivationFunctionType.Exp

    nc = bass.Bass()
    # host-prepped, bf16:
    xqT = nc.dram_tensor("xqt", [D, LQ], bf16, kind="ExternalInput")
    xkT = nc.dram_tensor("xkt", [D, L], bf16, kind="ExternalInput")
    # xvT tiled [k, mg, 128, m8, 128] : per (k, mg) one [128, 8, 128] row tile
    xvT = nc.dram_tensor("xvt", [NKB, 2, P, 8, P], bf16, kind="ExternalInput")
    # wq/wk tiled [k, 128, m, 128] : per k one [128, 8, 128] row tile
    wq = nc.dram_tensor("wq", [NKB, P, NHB, P], bf16, kind="ExternalInput")
    wk = nc.dram_tensor("wk", [NKB, P, NHB, P], bf16, kind="ExternalInput")
    wv = nc.dram_tensor("wv", [D, H * DV], bf16, kind="ExternalInput")
    wout = nc.dram_tensor("wout", [H * DV, D], bf16, kind="ExternalInput")
    out = nc.dram_tensor("out", [LQ, D], f32, kind="ExternalOutput")

    lp = nc.allow_low_precision(
        reason="bf16 matmul operands; accumulation stays fp32 in PSUM"
    )
    lp.__enter__()
    with tile.TileContext(nc) as tc:
        with (
            tc.tile_pool(name="persist", bufs=1) as persist,
            tc.tile_pool(name="xin", bufs=2) as xin,
            tc.tile_pool(name="attn", bufs=6) as attn_pool,
            tc.tile_pool(name="small", bufs=1) as small,
            tc.tile_pool(name="outp", bufs=3) as outp,
            tc.tile_pool(name="psum", bufs=1, space="PSUM") as psum,
            tc.tile_pool(name="dramp", bufs=2, space="DRAM") as dramp,
        ):
            # ---- persistent SBUF tensors (bf16) ----
            QT = persist.tile([P, NHB, LQ], bf16)  # 16 KB/part
            KT = persist.tile([P, NHB, L], bf16)  # 32 KB/part
            VA = persist.tile([P, NSB, H, DV + 1], bf16)  # V_aug, 32.5 KB/part
            HT = persist.tile([P, NHB, LQ], bf16)  # heads^T, 16 KB/part
            WQ = persist.tile([P, NKB, NHB, P], bf16)  # 16 KB/part
            WK = persist.tile([P, NKB, NHB, P], bf16)  # 16 KB/part
            WV = persist.tile([P, NKB, H * DV], bf16)  # 16 KB/part
            WO = persist.tile([P, NHB, D], bf16)  # 16 KB/part
            # Keep the sync queue free for the first xk chunk (K-proj is the
            # startup critical path): weights go on the scalar-engine HWDGE
            # queue (idle at startup) and the gpsimd SWDGE queue.
            for k in range(NKB):
                nc.gpsimd.dma_start(out=WV[:, k], in_=wv[ts(k, P), :])
            for k in range(NKB):
                nc.scalar.dma_start(out=WK[:, k], in_=wk[k])
            for k in range(NKB):
                nc.scalar.dma_start(out=WQ[:, k], in_=wq[k])
            for k in range(NKB):
                nc.gpsimd.dma_start(out=WO[:, k], in_=wout[ts(k, P), :])
            # ones column of V_aug: single strided memset
            nc.gpsimd.memset(VA[:, :, :, DV : DV + 1], 1.0)

            def k_proj(half):
                """KT[:, m, :] for pairs of `half` (m in 4*half..4*half+3)."""
                for nh in range(4):  # tok chunks of 512
                    xk = xin.tile([P, NKB, 512], bf16, tag="xk")
                    for k in range(NKB):
                        nc.sync.dma_start(
                            out=xk[:, k], in_=xkT[ts(k, P), ts(nh, 512)]
                        )
                    for m in range(4 * half, 4 * half + 4):
                        pt = psum.tile([P, 512], f32, tag="proj", bufs=2)
                        for k in range(NKB):
                            nc.tensor.matmul(
                                pt[:, :],
                                lhsT=WK[:, k, m, :],
                                rhs=xk[:, k, :],
                                start=(k == 0),
                                stop=(k == NKB - 1),
                            )
                        nc.vector.tensor_copy(KT[:, m, ts(nh, 512)], pt[:, :])

            def q_proj(half):
                for nh in range(2):  # tok chunks of 512
                    xq = xin.tile([P, NKB, 512], bf16, tag="xk")
                    for k in range(NKB):
                        nc.sync.dma_start(
                            out=xq[:, k], in_=xqT[ts(k, P), ts(nh, 512)]
                        )
                    for m in range(4 * half, 4 * half + 4):
                        pt = psum.tile([P, 512], f32, tag="proj", bufs=2)
                        for k in range(NKB):
                            nc.tensor.matmul(
                                pt[:, :],
                                lhsT=WQ[:, k, m, :],
                                rhs=xq[:, k, :],
                                start=(k == 0),
                                stop=(k == NKB - 1),
                            )
                        nc.vector.tensor_copy(QT[:, m, ts(nh, 512)], pt[:, :])

            def kq_proj_m(w_res, x_dram, dst, n_chunk, m):
                """Project one head-pair block m (x chunks re-streamed)."""
                for nh in range(n_chunk):
                    xt = xin.tile([P, NKB, 512], bf16, tag="xk")
                    for k in range(NKB):
                        nc.sync.dma_start(
                            out=xt[:, k], in_=x_dram[ts(k, P), ts(nh, 512)]
                        )
                    pt = psum.tile([P, 512], f32, tag="proj", bufs=2)
                    for k in range(NKB):
                        nc.tensor.matmul(
                            pt[:, :],
                            lhsT=w_res[:, k, m, :],
                            rhs=xt[:, k, :],
                            start=(k == 0),
                            stop=(k == NKB - 1),
                        )
                    nc.vector.tensor_copy(dst[:, m, ts(nh, 512)], pt[:, :])

            def v_proj(half):
                """VA[:, :, 8*half:8*half+8, 0:64] (8 heads of `half`)."""
                for mgq in range(4):  # tok-block groups of 4
                    mg, sub = divmod(mgq, 2)
                    xv = xin.tile([P, NKB, 4, P], bf16, tag="xk")
                    for k in range(NKB):
                        nc.sync.dma_start(
                            out=xv[:, k], in_=xvT[k, mg][:, sub * 4 : sub * 4 + 4, :]
                        )
                    for m4 in range(4):
                        pt = psum.tile([P, 512], f32, tag="proj", bufs=2)
                        for k in range(NKB):
                            nc.tensor.matmul(
                                pt[:, :],
                                lhsT=xv[:, k, m4, :],
                                rhs=WV[:, k, ts(half, 512)],
                                start=(k == 0),
                                stop=(k == NKB - 1),
                            )
                        nc.vector.tensor_copy(
                            VA[:, mgq * 4 + m4, half * 8 : half * 8 + 8, 0:DV],
                            pt.rearrange("p (h v) -> p h v", h=8),
                        )

            def attention(hb):
                """One head pair: heads 2hb (partitions 0-63 of block hb) and
                2hb+1 (partitions 64-127), all 16 s-blocks, both q halves."""
                for qh in range(2):
                    opt = psum.tile([P, 2, 512], f32, tag="op", bufs=1)
                    for s in range(NSB):
                        sp = psum.tile([P, 2, 512], f32, tag="sp", bufs=2)
                        # two concurrent row-tiled K=64 matmuls (head pair)
                        nc.tensor.matmul(
                            sp[:, 0, :],
                            lhsT=KT[0:DK, hb, ts(s, P)],
                            rhs=QT[0:DK, hb, ts(qh, 512)],
                            start=True,
                            stop=True,
                        )
                        nc.tensor.matmul(
                            sp[:, 1, :],
                            lhsT=KT[DK:P, hb, ts(s, P)],
                            rhs=QT[DK:P, hb, ts(qh, 512)],
                            start=True,
                            stop=True,
                        )
                        ae = attn_pool.tile([P, 2, 512], bf16, tag="ae")
                        nc.scalar.activation(
                            out=ae[:, :, :], in_=sp[:, :, :], func=EXP, scale=0.125
                        )
                        for j in range(2):
                            nc.tensor.matmul(
                                opt[0 : DV + 1, j, :],
                                lhsT=VA[:, s, 2 * hb + j, :],
                                rhs=ae[:, j, :],
                                start=(s == 0),
                                stop=(s == NSB - 1),
                            )
                    # ---- softmax epilogue for this (pair, qh) ----
                    # Free `opt` fast (it is single-buffered): extract the
                    # denominator rows (fp32) and the unnormalized heads
                    # (bf16), then normalize lazily off the critical path.
                    den = small.tile([1, 2, 512], f32, tag="den")
                    nc.vector.tensor_copy(den[:, :, :], opt[DV : DV + 1, :, :])
                    htu = small.tile([DV, 2, 512], bf16, tag="htu", bufs=2)
                    nc.vector.tensor_copy(htu[:, :, :], opt[0:DV, :, :])
                    dden = dramp.tile(
                        [1, 2, 512], f32, tag="dden", name=f"dden_{hb}_{qh}"
                    )
                    nc.gpsimd.dma_start(out=dden[:, :, :], in_=den[:, :, :])
                    den16 = small.tile([16, DV], f32, tag="den16")
                    nc.gpsimd.dma_start(
                        out=den16[:, :],
                        in_=dden.rearrange("p h (j c) -> p (h j) c", j=8)[0],
                    )
                    r16 = small.tile([16, DV], f32, tag="r16")
                    nc.vector.reciprocal(r16[:, :], den16[:, :])
                    rb16 = small.tile([16, DV], bf16, tag="rb16")
                    nc.vector.tensor_copy(rb16[:, :], r16[:, :])
                    rden = dramp.tile(
                        [1, 2, 512], bf16, tag="rden", name=f"rden_{hb}_{qh}"
                    )
                    nc.gpsimd.dma_start(
                        out=rden.rearrange("p h (j c) -> p (h j) c", j=8)[0],
                        in_=rb16[:, :],
                    )
                    rcb = small.tile([DV, 2, 512], bf16, tag="rcb")
                    nc.gpsimd.dma_start(
                        out=rcb[:, :, :],
                        in_=rden[0:1, :, :].to_broadcast((DV, 2, 512)),
                    )
                    for j in range(2):
                        nc.vector.tensor_mul(
                            HT[j * DK : j * DK + DK, hb, ts(qh, 512)],
                            htu[:, j, :],
                            rcb[:, j, :],
                        )

            # ---- phased schedule ----
            # attention(0) is emitted before v_proj(0): its scores/exp only
            # need KT/QT, so exp starts ~45us earlier; its attn@V matmuls
            # block on VA and the scheduler fills with v_proj work.
            k_proj(0)
            q_proj(0)
            v_proj(0)
            attention(0)
            # half-1 projections: emitted here (before the hb>=4 attention
            # passes -- the VA dependency tracker needs v_proj writes emitted
            # before the attn@V reads) but DEPRIORITIZED so the scheduler
            # prefers the exp critical chain and uses projections as filler.
            _save_pri = tc.cur_priority
            tc.cur_priority = 10_000_000
            k_proj(1)
            q_proj(1)
            v_proj(1)
            tc.cur_priority = _save_pri
            for hb in range(1, NHB):
                attention(hb)

            # ---- out projection ----
            for nh in range(2):  # dm halves
                for m in range(NMQ):
                    pt = psum.tile([P, 512], f32, tag="proj", bufs=2, name=f"po_{nh}_{m}")
                    for hb in range(NHB):
                        nc.tensor.matmul(
                            pt[:, :],
                            lhsT=HT[:, hb, ts(m, P)],
                            rhs=WO[:, hb, ts(nh, 512)],
                            start=(hb == 0),
                            stop=(hb == NHB - 1),
                        )
                    ot = outp.tile([P, 512], f32, tag="ot", name=f"ot_{nh}_{m}")
                    nc.vector.tensor_copy(ot, pt[:, :])
                    (nc.gpsimd if m % 2 == 0 else nc.sync).dma_start(
                        out=out[ts(m, P), ts(nh, 512)], in_=ot
                    )
    lp.__exit__(None, None, None)

    _split_multi_waits(nc)
    return nc


def _split_multi_waits(nc, max_waits: int = 1):
    """Walrus's setupSyncWait rejects instructions carrying more than a
    struct-specific number of sync waits (e.g. the Tile kernel-tail Drain
    gathers one wait per live semaphore). Hoist excess waits into prepended
    single-wait NoOps on the same engine."""
    from concourse import mybir

    for f in nc.m.functions:
        for blk in f.blocks:
            out = []
            for inst in blk.instructions:
                si = inst.sync_info
                waits = list(si.on_wait) if (si is not None and si.on_wait) else []
                if len(waits) > max_waits:
                    keep = waits[-max_waits:]
                    for w in waits[:-max_waits]:
                        nop = mybir.InstNoOp(
                            name=nc.get_next_instruction_name(),
                            ins=[],
                            outs=[],
                            sync_info=mybir.SyncInfo(on_wait=[w], on_update=[]),
                        )
                        nop.engine = inst.engine
                        try:
                            nop.bass_nofuse = True
                        except Exception:
                            pass
                        nc.register_instruction(nop)
                        out.append(nop)
                    si.on_wait = keep
                out.append(inst)
            blk.instructions = out


def _get_nc():
    if "nc" not in _CACHE:
        _CACHE["nc"] = _build_bass()
    return _CACHE["nc"]


def _prep_in_maps(x_query, x_key, x_value, Wq, Wk, Wv, Wout):
    import ml_dtypes

    bf = ml_dtypes.bfloat16
    x_query = np.asarray(x_query, dtype=np.float32)
    x_key = np.asarray(x_key, dtype=np.float32)
    x_value = np.asarray(x_value, dtype=np.float32)
    # [H, D, dk] -> [D, H*dk]
    wq_cat = np.asarray(Wq, np.float32).transpose(1, 0, 2).reshape(D, H * DK)
    wk_cat = np.asarray(Wk, np.float32).transpose(1, 0, 2).reshape(D, H * DK)
    wv_cat = np.ascontiguousarray(
        np.asarray(Wv, np.float32).transpose(1, 0, 2).reshape(D, H * DV)
    ).astype(bf)
    # wq/wk into [k, 128, m, 128] (contiguous [m,128] per (k,p) row)
    wq_t = np.ascontiguousarray(wq_cat.reshape(NKB, P, NHB, P)).astype(bf)
    wk_t = np.ascontiguousarray(wk_cat.reshape(NKB, P, NHB, P)).astype(bf)
    wout_c = np.ascontiguousarray(np.asarray(Wout, np.float32)).astype(bf)

    in_maps = []
    for c in range(8):
        b, half = divmod(c, 2)
        xq_sh = np.ascontiguousarray(
            x_query[b, half * LQ : (half + 1) * LQ, :].T
        ).astype(bf)  # [D, LQ]
        xk_sh = np.ascontiguousarray(x_key[b].T).astype(bf)  # [D, L]
        xvT_full = x_value[b].T  # [D, L]
        # [k, mg, 128, m8, 128]
        xv_t = np.ascontiguousarray(
            xvT_full.reshape(NKB, P, 2, 8, P).transpose(0, 2, 1, 3, 4)
        ).astype(bf)
        in_maps.append(
            {
                "xqt": xq_sh,
                "xkt": xk_sh,
                "xvt": xv_t,
                "wq": wq_t,
                "wk": wk_t,
                "wv": wv_cat,
                "wout": wout_c,
            }
        )
    return in_maps


def kernel(x_query, x_key, x_value, Wq, Wk, Wv, Wout):
    from concourse.bass_utils import run_bass_kernel_spmd

    nc = _get_nc()
    in_maps = _prep_in_maps(x_query, x_key, x_value, Wq, Wk, Wv, Wout)
    trace = bool(int(os.environ.get("MHA_TRACE", "0")))
    res = run_bass_kernel_spmd(nc, in_maps, list(range(8)), trace=trace)
    _CACHE["last_result"] = res
    out = np.empty((B, L, D), np.float32)
    for c in range(8):
        b, half = divmod(c, 2)
        out[b, half * LQ : (half + 1) * LQ, :] = res.results[c]["out"]
    return out
